# revision 27
# baseline (speedup 1.0000x reference)
"""Trainium2 Bass kernel for nn_DeformableUpdatingModel.

Math: the model output is only the spatial mean of a chain of linear ops, so
everything collapses:
  out[m,o] = (1/HW) * ( sum_i Wc[o,i] * g[m,i] + be2[o] * s_m ) + b_dc[o]
  Wc = W_dc @ W_emb,  be2 = W_dc @ b_emb
  g[m,i] = sum_q wsum_m[q] * IF[frame(m), i, q]
  s_m    = sum_q wsum_m[q]
  wsum_m[v,u] = sum_{r,s} tent(v - y_m(r,s)) * tent(u - x_m(r,s))
  tent(d) = relu(1 - |d|)   (zero-padding falls out since v,u in [0,64))
  y_m(r,s) = r + dy, x_m(r,s) = s + dx, (dy,dx) = 0.0625 * (2x2 pixel sums of
  p_motions at rows/cols {4k+1,4k+2})  [bilinear 4x downsample * 0.25 scale]

Device mapping per core (2 I-frames, 6 flows):
  - contraction tiling p = (r-parity, s): partition p -> r = 2k + p//64,
    s = p%64, free block k in [0,32).
  - tents built in fp16 from *centered* integer iotas (exact in fp16) minus
    the small dy/dx (|.| <= ~4.6), so no catastrophic quantization.
  - T-side tents are windowed: tent(v-y) support fits in a 16-aligned 32-wide
    v-window per k-region (valid while |flow| < 7; actual data max ~4.6).
  - wsum via PE: stationary U'(dup to 128 cols), moving T' -> PSUM(128,64)
    holds wsumT duplicated across partition halves; scattered into
    wsumt[p, 3*j2+m] so the final contraction is plain K=128 matmuls.
  - FpT[q,o] = F^T Wc^T via PE with F as bf16 stationary (cast during DMA);
    ones column appended for s_m.

Sharding: data-parallel over the 48 (B*num_gop*t) maps; core c owns the two
I-frames {2c, 2c+1} and their 6 flows.
"""
import sys
if '/opt/trn_rl_repo' not in sys.path:
    sys.path.insert(0, '/opt/trn_rl_repo')

import numpy as np

import concourse.bacc as bacc
import concourse.mybir as mybir
import concourse.tile as tile
from concourse.bass_utils import run_bass_kernel_spmd
from concourse.masks import make_identity

F32 = mybir.dt.float32
BF16 = mybir.dt.bfloat16
FP16 = mybir.dt.float16
I32 = mybir.dt.int32
U16 = mybir.dt.uint16
OP = mybir.AluOpType
ACT = mybir.ActivationFunctionType

B, T, GOP = 4, 16, 4
NUM_GOP = T // GOP
NFLOW = 48
C = 256
H = W = 64
HW = H * W
NCORES = 8
GOPS_PER_CORE = 2
FLOWS_PER_CORE = 6

# T-side v-window regions: (k_start, k_end, window_start)
TREG = [(0, 12, 0), (12, 20, 16), (20, 32, 32)]
TW = 32


def build_nc():
    nc = bacc.Bacc("TRN2", target_bir_lowering=False, debug=False,
                   num_devices=NCORES)

    d_if = nc.dram_tensor("ifeat", [GOPS_PER_CORE, C, HW], F32, kind="ExternalInput")
    d_pm = nc.dram_tensor("pmot", [FLOWS_PER_CORE, 2, 256, 256], F32, kind="ExternalInput")
    d_wemb = nc.dram_tensor("wemb", [C, C], F32, kind="ExternalInput")
    d_wdct = nc.dram_tensor("wdct", [C, C], F32, kind="ExternalInput")
    d_bemb3 = nc.dram_tensor("bemb3", [C, 3], F32, kind="ExternalInput")
    d_bdc3 = nc.dram_tensor("bdc3", [3, C], F32, kind="ExternalInput")
    d_out = nc.dram_tensor("out", [FLOWS_PER_CORE, C], F32, kind="ExternalOutput")

    with tile.TileContext(nc) as tc:
        with (
            tc.tile_pool(name="const", bufs=1) as cpool,
            tc.tile_pool(name="wpool", bufs=1) as wpool,
            tc.tile_pool(name="fpool", bufs=2) as fpool,
            tc.tile_pool(name="work", bufs=2) as work,
            tc.tile_pool(name="tu", bufs=2) as tupool,
            tc.tile_pool(name="tu1", bufs=1) as tupool1,
            tc.tile_pool(name="ps_fp", bufs=2, space="PSUM") as ps_fp,
            tc.tile_pool(name="ps_tr", bufs=2, space="PSUM") as ps_tr,
            tc.tile_pool(name="ps_w", bufs=2, space="PSUM") as ps_w,
            tc.tile_pool(name="ps_o", bufs=1, space="PSUM") as ps_o,
        ):
            # ---------------- constants ----------------
            ident = cpool.tile([64, 64], F32)
            make_identity(nc, ident)

            # iotw[p, k, w] = (wk + w) - 2k - p//64   (T-side windowed v iota)
            ioti = cpool.tile([128, 32, TW], I32)
            for (ka, kb, wk) in TREG:
                nc.gpsimd.iota(ioti[:, ka:kb, :],
                               pattern=[[-2, kb - ka], [1, TW]],
                               base=wk - 2 * ka, channel_multiplier=0)
            nc.vector.tensor_scalar(ioti[64:128, :, :], ioti[64:128, :, :], 1,
                                    None, op0=OP.subtract)
            iotw = cpool.tile([128, 32, TW], FP16)
            nc.vector.tensor_copy(iotw[:], ioti[:])

            # iotu[p, k, u] = u - p%64   (U-side full-width u iota)
            iotui = cpool.tile([128, 32, 64], I32)
            nc.gpsimd.iota(iotui[:], pattern=[[0, 32], [1, 64]], base=0,
                           channel_multiplier=-1)
            nc.vector.tensor_scalar(iotui[64:128, :, :], iotui[64:128, :, :], 64,
                                    None, op0=OP.add)
            iotu = cpool.tile([128, 32, 64], FP16)
            nc.scalar.copy(iotu[:], iotui[:])

            # ---------------- weights ----------------
            wemb = [wpool.tile([128, C], F32, tag=f"wemb{k}", name=f"wemb{k}") for k in range(2)]
            wdct = [wpool.tile([128, C], F32, tag=f"wdct{k}", name=f"wdct{k}") for k in range(2)]
            bemb3 = [wpool.tile([128, 3], F32, tag=f"bemb{k}", name=f"bemb{k}") for k in range(2)]
            for k in range(2):
                nc.sync.dma_start(wemb[k][:], d_wemb[128 * k:128 * (k + 1), :])
                nc.sync.dma_start(wdct[k][:], d_wdct[128 * k:128 * (k + 1), :])
                nc.sync.dma_start(bemb3[k][:], d_bemb3[128 * k:128 * (k + 1), :])
            bdc3 = wpool.tile([3, C], F32)
            nc.sync.dma_start(bdc3[:], d_bdc3[:])

            # WcT[i, o] = sum_c W_emb[c, i] * W_dcT[c, o]  (bf16 for Fp matmul)
            wct = [wpool.tile([128, C], BF16, tag=f"wct{k}", name=f"wct{k}") for k in range(2)]
            for mi in range(2):
                p = ps_fp.tile([128, 512], F32, tag="p")
                for kc in range(2):
                    nc.tensor.matmul(p[:, 0:256], wemb[kc][:, 128 * mi:128 * (mi + 1)],
                                     wdct[kc][:], start=(kc == 0), stop=(kc == 1))
                nc.vector.tensor_copy(wct[mi][:], p[:, 0:256])
            be2 = wpool.tile([3, C], F32)
            pb = ps_fp.tile([128, 512], F32, tag="p")
            for kc in range(2):
                nc.tensor.matmul(pb[0:3, 0:256], bemb3[kc][:], wdct[kc][:],
                                 start=(kc == 0), stop=(kc == 1))
            nc.vector.tensor_copy(be2[:], pb[0:3, 0:256])

            # FpT: one merged tile; ones column at v-index 256 of each slot
            fpt = cpool.tile([128, 32, 258], BF16)
            nc.vector.memset(fpt[:, :, 256:257], 1.0)

            # persistent zeroed T'-slots (windows overwritten every flow)
            tslots = [cpool.tile([128, 32, 64], FP16, tag=f"ts{i}", name=f"ts{i}")
                      for i in range(2)]
            for i in range(2):
                nc.gpsimd.memset(tslots[i][:], 0.0)

            # ---------------- per gop ----------------
            for g in range(GOPS_PER_CORE):
                fk = [fpool.tile([128, HW], BF16, tag=f"f{kc}", name=f"fk{kc}")
                      for kc in range(2)]
                for kc in range(2):
                    nc.gpsimd.dma_start(fk[kc][:], d_if[g, 128 * kc:128 * (kc + 1), :])

                # --- flow prep for all 3 flows first (feeds DVE/ACT early) ---
                ybigs, xbigs = [], []
                for mm in range(3):
                    fg = 3 * g + mm
                    pmv = d_pm[fg:fg + 1, :, :, :].squeeze(0) \
                        .rearrange("c (i f) w -> i c f w", f=4)
                    pt = work.tile([64, 2, 2, 256], F32, tag="pm", name=f"pt{fg}")
                    nc.sync.dma_start(pt[:], pmv[:, :, 1:3, :])

                    tA = work.tile([64, 2, 64], F32, tag="tA", name=f"tA{fg}")
                    tB = work.tile([64, 2, 64], F32, tag="tB", name=f"tB{fg}")
                    nc.vector.tensor_tensor(out=tA[:], in0=pt[:, :, 0:1, 1:254:4],
                                            in1=pt[:, :, 0:1, 2:255:4], op=OP.add)
                    nc.vector.tensor_tensor(out=tB[:], in0=pt[:, :, 1:2, 1:254:4],
                                            in1=pt[:, :, 1:2, 2:255:4], op=OP.add)
                    nc.vector.tensor_tensor(out=tA[:], in0=tA[:], in1=tB[:],
                                            op=OP.add)
                    ds2 = work.tile([64, 2, 64], F32, tag="ds2", name=f"ds2{fg}")
                    nc.vector.tensor_scalar(ds2[:], tA[:], 0.0625, None,
                                            op0=OP.mult)
                    # duplicated (64,128) stationaries for the transpose matmul
                    dsy = work.tile([64, 2, 64], F32, tag="dsy", name=f"dsy{fg}")
                    dsx = work.tile([64, 2, 64], F32, tag="dsx", name=f"dsx{fg}")
                    nc.scalar.copy(dsy[:], ds2[:, 0:1, :].broadcast_to([64, 2, 64]))
                    nc.scalar.copy(dsx[:], ds2[:, 1:2, :].broadcast_to([64, 2, 64]))

                    # transpose: ptr[p, r] = dy[r, p%64]  (both halves)
                    ptr = ps_tr.tile([128, 64], F32, tag="ptr", name=f"ptry{fg}")
                    nc.tensor.matmul(ptr[:], dsy[:], ident[:], start=True, stop=True)
                    ptr2 = ps_tr.tile([128, 64], F32, tag="ptr", name=f"ptrx{fg}")
                    nc.tensor.matmul(ptr2[:], dsx[:], ident[:], start=True, stop=True)
                    # ybig[p, k] = dy(r=2k+p//64, s=p%64)  (fp16)
                    ybig = work.tile([128, 32], FP16, tag="ybig", name=f"yb{fg}")
                    nc.scalar.copy(ybig[0:64, :], ptr[0:64, 0:64:2])
                    nc.scalar.copy(ybig[64:128, :], ptr[64:128, 1:64:2])
                    xbig = work.tile([128, 32], FP16, tag="xbig", name=f"xb{fg}")
                    nc.scalar.copy(xbig[0:64, :], ptr2[0:64, 0:64:2])
                    nc.scalar.copy(xbig[64:128, :], ptr2[64:128, 1:64:2])
                    ybigs.append(ybig)
                    xbigs.append(xbig)

                # --- Fp production (PE) overlaps tent construction (DVE) ---
                for j2 in range(0, 32, 2):
                    p = ps_fp.tile([128, 512], F32, tag="p", name=f"p{g}_{j2}")
                    for jj in range(2):
                        for kc in range(2):
                            nc.tensor.matmul(
                                p[:, 256 * jj:256 * (jj + 1)],
                                fk[kc][:, 128 * (j2 + jj):128 * (j2 + jj + 1)],
                                wct[kc][:], start=(kc == 0), stop=(kc == 1))
                    nc.scalar.copy(fpt[:, j2:j2 + 2, 0:256],
                                   p[:].rearrange("p (a b) -> p a b", b=256))

                wsumt = work.tile([128, 96], BF16, tag="wsumt", name=f"ws{g}")

                # --- tents + wsum per flow ---
                for mm in range(3):
                    fg = 3 * g + mm
                    ybig, xbig = ybigs[mm], xbigs[mm]

                    # T side (windowed): d = iotw - dy
                    dT = tupool.tile([128, 32, TW], FP16, tag="dt", name=f"dt{fg}")
                    nc.vector.tensor_tensor(
                        out=dT[:], in0=iotw[:],
                        in1=ybig[:].unsqueeze(2).broadcast_to([128, 32, TW]),
                        op=OP.subtract)
                    mT = tupool1.tile([128, 32, TW], FP16, tag="mt", name=f"mt{fg}")
                    nc.vector.tensor_scalar(mT[:].bitcast(U16), dT[:].bitcast(U16),
                                            0x7FFF, None, op0=OP.bitwise_and)
                    tsl = tslots[fg % 2]
                    for (ka, kb, wk) in TREG:
                        nc.vector.tensor_scalar(tsl[:, ka:kb, wk:wk + TW],
                                                mT[:, ka:kb, :], 1.0, 1.0,
                                                op0=OP.min, op1=OP.subtract)

                    # U side (full width, duplicated): d = iotu - dx
                    dU = tupool.tile([128, 32, 64], FP16, tag="du", name=f"du{fg}")
                    if mm == 0:
                        nc.gpsimd.tensor_tensor(
                            out=dU[:], in0=iotu[:],
                            in1=xbig[:].unsqueeze(2).broadcast_to([128, 32, 64]),
                            op=OP.subtract)
                    else:
                        nc.vector.tensor_tensor(
                            out=dU[:], in0=iotu[:],
                            in1=xbig[:].unsqueeze(2).broadcast_to([128, 32, 64]),
                            op=OP.subtract)
                    mU = tupool1.tile([128, 32, 64], FP16, tag="mu", name=f"mu{fg}")
                    nc.vector.tensor_scalar(mU[:].bitcast(U16), dU[:].bitcast(U16),
                                            0x7FFF, None, op0=OP.bitwise_and)
                    ub = tupool.tile([128, 32, 128], FP16, tag="bu", name=f"ub{fg}")
                    nc.vector.tensor_scalar(ub[:, :, 0:64], mU[:], 1.0, 1.0,
                                            op0=OP.min, op1=OP.subtract)
                    nc.vector.tensor_scalar(ub[:, :, 64:128], mU[:], 1.0, 1.0,
                                            op0=OP.min, op1=OP.subtract)

                    # wsumT (dup) = sum_p U'[p, u] T'[p, v]
                    pw = ps_w.tile([128, 64], F32, tag="pw", name=f"pw{fg}")
                    for k in range(32):
                        nc.tensor.matmul(pw[:], ub[:, k:k + 1, :],
                                         tsl[:, k:k + 1, :],
                                         start=(k == 0), stop=(k == 31))
                    nc.scalar.copy(wsumt[0:64, mm:96:3], pw[0:64, 0:64:2])
                    nc.scalar.copy(wsumt[64:128, mm:96:3], pw[64:128, 1:64:2])

                # --- final contraction ---
                po = ps_o.tile([3, 257], F32, tag="po", name=f"po{g}")
                for j2 in range(32):
                    nc.tensor.matmul(po[:], wsumt[:, 3 * j2:3 * (j2 + 1)],
                                     fpt[:, j2:j2 + 1, 0:257],
                                     start=(j2 == 0), stop=(j2 == 31))

                aux = work.tile([3, C], F32, tag="aux", name=f"aux{g}")
                nc.vector.tensor_scalar(aux[:], be2[:], po[:, 256:257], None,
                                        op0=OP.mult)
                nc.vector.tensor_tensor(out=aux[:], in0=aux[:], in1=po[:, 0:256],
                                        op=OP.add)
                osb = work.tile([3, C], F32, tag="osb", name=f"osb{g}")
                nc.vector.scalar_tensor_tensor(osb[:], in0=aux[:],
                                               scalar=1.0 / HW, in1=bdc3[:],
                                               op0=OP.mult, op1=OP.add)
                nc.sync.dma_start(d_out[3 * g:3 * (g + 1), :], osb[:])

    nc.compile()
    return nc


_NC_CACHE = {}


def _get_nc():
    if "nc" not in _NC_CACHE:
        _NC_CACHE["nc"] = build_nc()
    return _NC_CACHE["nc"]


def make_in_maps(i_features, p_motions, W_emb, b_emb, W_dc, b_dc):
    i_features = np.ascontiguousarray(i_features, np.float32).reshape(16, C, HW)
    pm = np.ascontiguousarray(p_motions, np.float32).reshape(NFLOW, 2, 256, 256)
    wemb = np.ascontiguousarray(W_emb, np.float32)
    wdct = np.ascontiguousarray(np.asarray(W_dc, np.float32).T)
    bemb3 = np.ascontiguousarray(
        np.repeat(np.asarray(b_emb, np.float32)[:, None], 3, axis=1))
    bdc3 = np.ascontiguousarray(
        np.repeat(np.asarray(b_dc, np.float32)[None, :], 3, axis=0))
    in_maps = []
    for c in range(NCORES):
        in_maps.append({
            "ifeat": np.ascontiguousarray(i_features[2 * c:2 * c + 2]),
            "pmot": np.ascontiguousarray(pm[6 * c:6 * c + 6]),
            "wemb": wemb,
            "wdct": wdct,
            "bemb3": bemb3,
            "bdc3": bdc3,
        })
    return in_maps


def kernel(imgs, i_features, p_motions, W_emb, b_emb, W_dc, b_dc, _trace=False):
    nc = _get_nc()
    in_maps = make_in_maps(np.asarray(i_features), np.asarray(p_motions),
                           np.asarray(W_emb), np.asarray(b_emb),
                           np.asarray(W_dc), np.asarray(b_dc))
    res = run_bass_kernel_spmd(nc, in_maps, core_ids=list(range(NCORES)),
                               trace=_trace)
    out = np.concatenate([np.asarray(r["out"]) for r in res.results], axis=0)
    out = out.reshape(B, NUM_GOP, GOP - 1, C)
    if _trace:
        return out, res
    return out


# revision 28
# speedup vs baseline: 1.1341x; 1.1341x over previous
"""Trainium2 Bass kernel for nn_DeformableUpdatingModel.

Math: the model output is only the spatial mean of a chain of linear ops, so
everything collapses:
  out[m,o] = (1/HW) * ( sum_i Wc[o,i] * g[m,i] + be2[o] * s_m ) + b_dc[o]
  Wc = W_dc @ W_emb,  be2 = W_dc @ b_emb
  g[m,i] = sum_q wsum_m[q] * IF[frame(m), i, q]
  s_m    = sum_q wsum_m[q]
  wsum_m[v,u] = sum_{r,s} tent(v - y_m(r,s)) * tent(u - x_m(r,s))
  tent(d) = relu(1 - |d|)   (zero-padding falls out since v,u in [0,64))
  y_m(r,s) = r + dy, x_m(r,s) = s + dx, (dy,dx) = 0.0625 * (2x2 pixel sums of
  p_motions at rows/cols {4k+1,4k+2})  [bilinear 4x downsample * 0.25 scale]

Device mapping per core (2 I-frames, 6 flows):
  - contraction tiling p = (r-parity, s): partition p -> r = 2k + p//64,
    s = p%64, free block k in [0,32).
  - tents built in fp16 from *centered* integer iotas (exact in fp16) minus
    the small dy/dx (|.| <= ~4.6), so no catastrophic quantization.
  - T-side tents are windowed: tent(v-y) support fits in a 16-aligned 32-wide
    v-window per k-region (valid while |flow| < 7; actual data max ~4.6).
  - wsum via PE: stationary U'(dup to 128 cols), moving T' -> PSUM(128,64)
    holds wsumT duplicated across partition halves; scattered into
    wsumt[p, 3*j2+m] so the final contraction is plain K=128 matmuls.
  - FpT[q,o] = F^T Wc^T via PE with F as bf16 stationary (cast during DMA);
    ones column appended for s_m.

Sharding: data-parallel over the 48 (B*num_gop*t) maps; core c owns the two
I-frames {2c, 2c+1} and their 6 flows.
"""
import sys
if '/opt/trn_rl_repo' not in sys.path:
    sys.path.insert(0, '/opt/trn_rl_repo')

import numpy as np

import concourse.bacc as bacc
import concourse.mybir as mybir
import concourse.tile as tile
from concourse.bass_utils import run_bass_kernel_spmd
from concourse.masks import make_identity

F32 = mybir.dt.float32
BF16 = mybir.dt.bfloat16
FP16 = mybir.dt.float16
I32 = mybir.dt.int32
U16 = mybir.dt.uint16
OP = mybir.AluOpType
ACT = mybir.ActivationFunctionType

B, T, GOP = 4, 16, 4
NUM_GOP = T // GOP
NFLOW = 48
C = 256
H = W = 64
HW = H * W
NCORES = 8
GOPS_PER_CORE = 2
FLOWS_PER_CORE = 6

# T-side v-window regions: (k_start, k_end, window_start)
TREG = [(0, 12, 0), (12, 20, 16), (20, 32, 32)]
TW = 32


def build_nc():
    nc = bacc.Bacc("TRN2", target_bir_lowering=False, debug=False,
                   num_devices=NCORES)

    d_if = nc.dram_tensor("ifeat", [GOPS_PER_CORE, C, HW], F32, kind="ExternalInput")
    d_pm = nc.dram_tensor("pmot", [FLOWS_PER_CORE, 2, 256, 256], F32, kind="ExternalInput")
    d_wemb = nc.dram_tensor("wemb", [C, C], F32, kind="ExternalInput")
    d_wdct = nc.dram_tensor("wdct", [C, C], F32, kind="ExternalInput")
    d_bemb3 = nc.dram_tensor("bemb3", [C, 3], F32, kind="ExternalInput")
    d_bdc3 = nc.dram_tensor("bdc3", [3, C], F32, kind="ExternalInput")
    d_out = nc.dram_tensor("out", [FLOWS_PER_CORE, C], F32, kind="ExternalOutput")

    with tile.TileContext(nc) as tc:
        with (
            tc.tile_pool(name="const", bufs=1) as cpool,
            tc.tile_pool(name="wpool", bufs=1) as wpool,
            tc.tile_pool(name="fpool", bufs=2) as fpool,
            tc.tile_pool(name="work", bufs=3) as work,
            tc.tile_pool(name="tu", bufs=2) as tupool,
            tc.tile_pool(name="tu1", bufs=2) as tupool1,
            tc.tile_pool(name="ps_fp", bufs=2, space="PSUM") as ps_fp,
            tc.tile_pool(name="ps_tr", bufs=2, space="PSUM") as ps_tr,
            tc.tile_pool(name="ps_w", bufs=2, space="PSUM") as ps_w,
            tc.tile_pool(name="ps_o", bufs=1, space="PSUM") as ps_o,
        ):
            # ---------------- constants ----------------
            ident = cpool.tile([64, 64], F32)
            make_identity(nc, ident)

            # iotw[p, k, w] = (wk + w) - 2k - p//64   (T-side windowed v iota)
            ioti = cpool.tile([128, 32, TW], I32)
            for (ka, kb, wk) in TREG:
                nc.gpsimd.iota(ioti[:, ka:kb, :],
                               pattern=[[-2, kb - ka], [1, TW]],
                               base=wk - 2 * ka, channel_multiplier=0)
            nc.vector.tensor_scalar(ioti[64:128, :, :], ioti[64:128, :, :], 1,
                                    None, op0=OP.subtract)
            iotw = cpool.tile([128, 32, TW], FP16)
            nc.vector.tensor_copy(iotw[:], ioti[:])

            # iotu[p, k, u] = u - p%64   (U-side full-width u iota)
            iotui = cpool.tile([128, 32, 64], I32)
            nc.gpsimd.iota(iotui[:], pattern=[[0, 32], [1, 64]], base=0,
                           channel_multiplier=-1)
            nc.vector.tensor_scalar(iotui[64:128, :, :], iotui[64:128, :, :], 64,
                                    None, op0=OP.add)
            iotu = cpool.tile([128, 32, 64], FP16)
            nc.scalar.copy(iotu[:], iotui[:])

            # ---------------- weights ----------------
            wemb = [wpool.tile([128, C], F32, tag=f"wemb{k}", name=f"wemb{k}") for k in range(2)]
            wdct = [wpool.tile([128, C], F32, tag=f"wdct{k}", name=f"wdct{k}") for k in range(2)]
            bemb3 = [wpool.tile([128, 3], F32, tag=f"bemb{k}", name=f"bemb{k}") for k in range(2)]
            for k in range(2):
                nc.sync.dma_start(wemb[k][:], d_wemb[128 * k:128 * (k + 1), :])
                nc.sync.dma_start(wdct[k][:], d_wdct[128 * k:128 * (k + 1), :])
                nc.sync.dma_start(bemb3[k][:], d_bemb3[128 * k:128 * (k + 1), :])
            bdc3 = wpool.tile([3, C], F32)
            nc.sync.dma_start(bdc3[:], d_bdc3[:])

            # WcT[i, o] = sum_c W_emb[c, i] * W_dcT[c, o]  (bf16 for Fp matmul)
            wct = [wpool.tile([128, C], BF16, tag=f"wct{k}", name=f"wct{k}") for k in range(2)]
            for mi in range(2):
                p = ps_fp.tile([128, 512], F32, tag="p")
                for kc in range(2):
                    nc.tensor.matmul(p[:, 0:256], wemb[kc][:, 128 * mi:128 * (mi + 1)],
                                     wdct[kc][:], start=(kc == 0), stop=(kc == 1))
                nc.vector.tensor_copy(wct[mi][:], p[:, 0:256])
            be2 = wpool.tile([3, C], F32)
            pb = ps_fp.tile([128, 512], F32, tag="p")
            for kc in range(2):
                nc.tensor.matmul(pb[0:3, 0:256], bemb3[kc][:], wdct[kc][:],
                                 start=(kc == 0), stop=(kc == 1))
            nc.vector.tensor_copy(be2[:], pb[0:3, 0:256])

            # FpT: one merged tile; ones column at v-index 256 of each slot
            fpt = cpool.tile([128, 32, 258], BF16)
            nc.vector.memset(fpt[:, :, 256:257], 1.0)

            # persistent zeroed T'-slots (windows overwritten every flow)
            tslots = [cpool.tile([128, 32, 64], FP16, tag=f"ts{i}", name=f"ts{i}")
                      for i in range(3)]
            for i in range(3):
                nc.gpsimd.memset(tslots[i][:], 0.0)

            # ---------------- per gop ----------------
            for g in range(GOPS_PER_CORE):
                fk = [fpool.tile([128, HW], BF16, tag=f"f{kc}", name=f"fk{kc}")
                      for kc in range(2)]
                for kc in range(2):
                    nc.gpsimd.dma_start(fk[kc][:], d_if[g, 128 * kc:128 * (kc + 1), :])

                # --- flow prep for all 3 flows first (feeds DVE/ACT early) ---
                ybigs, xbigs = [], []
                for mm in range(3):
                    fg = 3 * g + mm
                    pmv = d_pm[fg:fg + 1, :, :, :].squeeze(0) \
                        .rearrange("c (i f) w -> i c f w", f=4)
                    pt = work.tile([64, 2, 2, 256], F32, tag="pm", name=f"pt{fg}")
                    nc.sync.dma_start(pt[:], pmv[:, :, 1:3, :])

                    tA = work.tile([64, 2, 64], F32, tag="tA", name=f"tA{fg}")
                    tB = work.tile([64, 2, 64], F32, tag="tB", name=f"tB{fg}")
                    nc.vector.tensor_tensor(out=tA[:], in0=pt[:, :, 0:1, 1:254:4],
                                            in1=pt[:, :, 0:1, 2:255:4], op=OP.add)
                    nc.vector.tensor_tensor(out=tB[:], in0=pt[:, :, 1:2, 1:254:4],
                                            in1=pt[:, :, 1:2, 2:255:4], op=OP.add)
                    nc.vector.tensor_tensor(out=tA[:], in0=tA[:], in1=tB[:],
                                            op=OP.add)
                    ds2 = work.tile([64, 2, 64], F32, tag="ds2", name=f"ds2{fg}")
                    nc.vector.tensor_scalar(ds2[:], tA[:], 0.0625, None,
                                            op0=OP.mult)
                    # duplicated (64,128) stationaries for the transpose matmul
                    dsy = work.tile([64, 2, 64], F32, tag="dsy", name=f"dsy{fg}")
                    dsx = work.tile([64, 2, 64], F32, tag="dsx", name=f"dsx{fg}")
                    nc.scalar.copy(dsy[:], ds2[:, 0:1, :].broadcast_to([64, 2, 64]))
                    nc.scalar.copy(dsx[:], ds2[:, 1:2, :].broadcast_to([64, 2, 64]))

                    # transpose: ptr[p, r] = dy[r, p%64]  (both halves)
                    ptr = ps_tr.tile([128, 64], F32, tag="ptr", name=f"ptry{fg}")
                    nc.tensor.matmul(ptr[:], dsy[:], ident[:], start=True, stop=True)
                    ptr2 = ps_tr.tile([128, 64], F32, tag="ptr", name=f"ptrx{fg}")
                    nc.tensor.matmul(ptr2[:], dsx[:], ident[:], start=True, stop=True)
                    # ybig[p, k] = dy(r=2k+p//64, s=p%64)  (fp16)
                    ybig = work.tile([128, 32], FP16, tag="ybig", name=f"yb{fg}")
                    nc.scalar.copy(ybig[0:64, :], ptr[0:64, 0:64:2])
                    nc.scalar.copy(ybig[64:128, :], ptr[64:128, 1:64:2])
                    xbig = work.tile([128, 32], FP16, tag="xbig", name=f"xb{fg}")
                    nc.scalar.copy(xbig[0:64, :], ptr2[0:64, 0:64:2])
                    nc.scalar.copy(xbig[64:128, :], ptr2[64:128, 1:64:2])
                    ybigs.append(ybig)
                    xbigs.append(xbig)

                # --- Fp production (PE) overlaps tent construction (DVE) ---
                for j2 in range(0, 32, 2):
                    p = ps_fp.tile([128, 512], F32, tag="p", name=f"p{g}_{j2}")
                    for jj in range(2):
                        for kc in range(2):
                            nc.tensor.matmul(
                                p[:, 256 * jj:256 * (jj + 1)],
                                fk[kc][:, 128 * (j2 + jj):128 * (j2 + jj + 1)],
                                wct[kc][:], start=(kc == 0), stop=(kc == 1))
                    nc.scalar.copy(fpt[:, j2:j2 + 2, 0:256],
                                   p[:].rearrange("p (a b) -> p a b", b=256))

                wsumt = work.tile([128, 96], BF16, tag="wsumt", name=f"ws{g}")

                # --- tents + wsum per flow ---
                for mm in range(3):
                    fg = 3 * g + mm
                    ybig, xbig = ybigs[mm], xbigs[mm]

                    # T side (windowed): d = iotw - dy
                    dT = tupool.tile([128, 32, TW], FP16, tag="dt", name=f"dt{fg}")
                    eng_dt = nc.gpsimd if mm == 1 else nc.vector
                    eng_dt.tensor_tensor(
                        out=dT[:], in0=iotw[:],
                        in1=ybig[:].unsqueeze(2).broadcast_to([128, 32, TW]),
                        op=OP.subtract)
                    mT = tupool1.tile([128, 32, TW], FP16, tag="mt", name=f"mt{fg}")
                    nc.vector.tensor_scalar(mT[:].bitcast(U16), dT[:].bitcast(U16),
                                            0x7FFF, None, op0=OP.bitwise_and)
                    tsl = tslots[fg % 3]
                    for (ka, kb, wk) in TREG:
                        nc.vector.tensor_scalar(tsl[:, ka:kb, wk:wk + TW],
                                                mT[:, ka:kb, :], 1.0, 1.0,
                                                op0=OP.min, op1=OP.subtract)

                    # U side (full width, duplicated): d = iotu - dx
                    dU = tupool.tile([128, 32, 64], FP16, tag="du", name=f"du{fg}")
                    if mm == 0:
                        nc.gpsimd.tensor_tensor(
                            out=dU[:], in0=iotu[:],
                            in1=xbig[:].unsqueeze(2).broadcast_to([128, 32, 64]),
                            op=OP.subtract)
                    else:
                        nc.vector.tensor_tensor(
                            out=dU[:], in0=iotu[:],
                            in1=xbig[:].unsqueeze(2).broadcast_to([128, 32, 64]),
                            op=OP.subtract)
                    mU = tupool1.tile([128, 32, 64], FP16, tag="mu", name=f"mu{fg}")
                    nc.vector.tensor_scalar(mU[:].bitcast(U16), dU[:].bitcast(U16),
                                            0x7FFF, None, op0=OP.bitwise_and)
                    ub = tupool.tile([128, 32, 128], FP16, tag="bu", name=f"ub{fg}")
                    nc.vector.tensor_scalar(ub[:, :, 0:64], mU[:], 1.0, 1.0,
                                            op0=OP.min, op1=OP.subtract)
                    nc.vector.tensor_scalar(ub[:, :, 64:128], mU[:], 1.0, 1.0,
                                            op0=OP.min, op1=OP.subtract)

                    # wsumT (dup) = sum_p U'[p, u] T'[p, v]
                    pw = ps_w.tile([128, 64], F32, tag="pw", name=f"pw{fg}")
                    for k in range(32):
                        nc.tensor.matmul(pw[:], ub[:, k:k + 1, :],
                                         tsl[:, k:k + 1, :],
                                         start=(k == 0), stop=(k == 31))
                    nc.scalar.copy(wsumt[0:64, mm:96:3], pw[0:64, 0:64:2])
                    nc.scalar.copy(wsumt[64:128, mm:96:3], pw[64:128, 1:64:2])

                # --- final contraction ---
                po = ps_o.tile([3, 257], F32, tag="po", name=f"po{g}")
                for j2 in range(32):
                    nc.tensor.matmul(po[:], wsumt[:, 3 * j2:3 * (j2 + 1)],
                                     fpt[:, j2:j2 + 1, 0:257],
                                     start=(j2 == 0), stop=(j2 == 31))

                aux = work.tile([3, C], F32, tag="aux", name=f"aux{g}")
                nc.vector.tensor_scalar(aux[:], be2[:], po[:, 256:257], None,
                                        op0=OP.mult)
                nc.vector.tensor_tensor(out=aux[:], in0=aux[:], in1=po[:, 0:256],
                                        op=OP.add)
                osb = work.tile([3, C], F32, tag="osb", name=f"osb{g}")
                nc.vector.scalar_tensor_tensor(osb[:], in0=aux[:],
                                               scalar=1.0 / HW, in1=bdc3[:],
                                               op0=OP.mult, op1=OP.add)
                nc.sync.dma_start(d_out[3 * g:3 * (g + 1), :], osb[:])

    nc.compile()
    return nc


_NC_CACHE = {}


def _get_nc():
    if "nc" not in _NC_CACHE:
        _NC_CACHE["nc"] = build_nc()
    return _NC_CACHE["nc"]


def make_in_maps(i_features, p_motions, W_emb, b_emb, W_dc, b_dc):
    i_features = np.ascontiguousarray(i_features, np.float32).reshape(16, C, HW)
    pm = np.ascontiguousarray(p_motions, np.float32).reshape(NFLOW, 2, 256, 256)
    wemb = np.ascontiguousarray(W_emb, np.float32)
    wdct = np.ascontiguousarray(np.asarray(W_dc, np.float32).T)
    bemb3 = np.ascontiguousarray(
        np.repeat(np.asarray(b_emb, np.float32)[:, None], 3, axis=1))
    bdc3 = np.ascontiguousarray(
        np.repeat(np.asarray(b_dc, np.float32)[None, :], 3, axis=0))
    in_maps = []
    for c in range(NCORES):
        in_maps.append({
            "ifeat": np.ascontiguousarray(i_features[2 * c:2 * c + 2]),
            "pmot": np.ascontiguousarray(pm[6 * c:6 * c + 6]),
            "wemb": wemb,
            "wdct": wdct,
            "bemb3": bemb3,
            "bdc3": bdc3,
        })
    return in_maps


def kernel(imgs, i_features, p_motions, W_emb, b_emb, W_dc, b_dc, _trace=False):
    nc = _get_nc()
    in_maps = make_in_maps(np.asarray(i_features), np.asarray(p_motions),
                           np.asarray(W_emb), np.asarray(b_emb),
                           np.asarray(W_dc), np.asarray(b_dc))
    res = run_bass_kernel_spmd(nc, in_maps, core_ids=list(range(NCORES)),
                               trace=_trace)
    out = np.concatenate([np.asarray(r["out"]) for r in res.results], axis=0)
    out = out.reshape(B, NUM_GOP, GOP - 1, C)
    if _trace:
        return out, res
    return out
